# revision 12
# baseline (speedup 1.0000x reference)
"""MoRALayer Trainium2 kernel.

Data-parallel over 8 NeuronCores (2048 samples each). Per core, feature-major
layout ([feature, sample] tiles): 10-step LSTM ACT loop, 2 sparse-MoE levels
(top-2 of 4 experts, computed densely), LayerNorm.

Precision scheme: weights feed single-pass float32r matmuls (PE truncates to
FP22, ~6e-5 relative). Activations on the routing-critical path (x, ACT avg,
level-0 expert mids/outputs) are carried as hi/lo f32r pairs so routing logits
are ~fp32-exact; level-1 experts run single-pass (their smooth noise is
post-routing). Elementwise math and transcendentals are fp32.
"""
import os
import sys
import numpy as np

for p in ("/root/.axon_site", "/root/.axon_site/_ro/trn_rl_repo",
          "/root/.axon_site/_ro/pypackages", "/opt/trn_rl_repo"):
    if os.path.isdir(p) and p not in sys.path:
        sys.path.append(p)

import concourse.bass as bass
import concourse.mybir as mybir
import concourse.tile as tile
from concourse import bacc

dt = mybir.dt
AF = mybir.ActivationFunctionType
ALU = mybir.AluOpType
AX = mybir.AxisListType

B, D, H, E, LV, TOPK, T = 16384, 768, 768, 4, 2, 2, 10
LN_EPS = 1e-5
NCORES = 8
BPC = B // NCORES          # samples per core
ABS = 512                  # samples per block (A1 phase)
NBS = 256                  # samples per block (A2 LSTM phase)
MBS = 256                  # samples per block (MoE expert pass)
GBS = 512                  # samples per block (MoE gates pass)
KC = D // 128              # 6 k-chunks over features
GM = 4 * H // 128          # 24 gate m-chunks

F32, F32R = dt.float32, dt.float32r


def build(reps=1):
    nc = bacc.Bacc("TRN2", target_bir_lowering=False, debug=False)
    names = []

    def inp(name, shape, dtyp=F32):
        names.append(name)
        return nc.dram_tensor(name, list(shape), dtyp, kind="ExternalInput").ap()

    # inputs (weights host-transposed, raw fp32 bits fed as float32r)
    x_d = inp("x", [BPC, D])
    wi_d = inp("wiT", [D, 4 * H], F32R)
    wh_d = inp("whT", [D, 4 * H], F32R)
    whalt_d = inp("whaltT", [D, 1], F32R)
    bib_d = inp("bib", [4 * H, 1])         # bi + bh
    bhalt_d = inp("bhalt", [1, 1])
    gate_d = inp("gateT", [LV, D, E], F32R)
    gateb_d = inp("gateb", [LV, 1, E])
    w1_d = inp("w1T", [LV, E, H, H], F32R)
    w2_d = inp("w2T", [LV, E, H, D], F32R)
    b1_d = inp("b1", [LV, E, H, 1])
    b2s_d = inp("b2s", [LV, E, D])         # stacked for the bias matmul
    gamma_d = inp("gamma", [D, 1])
    beta_d = inp("beta", [D, 1])
    ident_d = inp("ident128", [128, 128])
    out_d = nc.dram_tensor("out", [BPC, D], F32, kind="ExternalOutput").ap()

    # DRAM scratch (per-core activations, feature-major)
    xwi_d = nc.dram_tensor("xwi_s", [KC, 4, 128, BPC], F32).ap()
    avgh_d = nc.dram_tensor("avgh_s", [D, BPC], F32R).ap()
    avgl_d = nc.dram_tensor("avgl_s", [D, BPC], F32R).ap()
    l0h_d = nc.dram_tensor("l0h_s", [D, BPC], F32R).ap()
    l0l_d = nc.dram_tensor("l0l_s", [D, BPC], F32R).ap()
    l1o_d = nc.dram_tensor("l1o_s", [D, BPC], F32).ap()

    with tile.TileContext(nc) as tc:
      for rep in range(reps):
        R = f"r{rep}_"
        with tc.tile_pool(name=R + "const", bufs=1) as cpool:
            ident = cpool.tile([128, 128], F32, tag="ident")
            nc.sync.dma_start(out=ident[:], in_=ident_d[:])
            ones1 = cpool.tile([1, 128], F32, tag="ones1")
            nc.vector.memset(ones1[:], 1.0)
            cst01 = cpool.tile([1, 128], F32, tag="cst01")
            nc.vector.memset(cst01[:], 1.0 / T)
            oinv = cpool.tile([128, 1], F32, tag="oinv")
            nc.vector.memset(oinv[:], 1.0 / D)
            eps_t = cpool.tile([1, 1], F32, tag="epsln")
            nc.vector.memset(eps_t[:], LN_EPS)
            bhalt_t = cpool.tile([1, 1], F32, tag="bhalt")
            nc.sync.dma_start(out=bhalt_t[:], in_=bhalt_d[:, :])
            whalt_t = []
            for k in range(KC):
                t = cpool.tile([128, 1], F32R, tag=f"whalt{k}")
                nc.sync.dma_start(out=t[:], in_=whalt_d[k * 128:(k + 1) * 128, :])
                whalt_t.append(t)

            # ================= phase A1: xWi (x split hi/lo) =================
            with tc.tile_pool(name=R + "wiA", bufs=1) as wpool, \
                 tc.tile_pool(name=R + "a1work", bufs=2) as work, \
                 tc.tile_pool(name=R + "a1ps", bufs=3, space="PSUM") as pspool, \
                 tc.tile_pool(name=R + "a1pt", bufs=2, space="PSUM") as ptpool:
                wi_t = []
                for k in range(KC):
                    t = wpool.tile([128, 4 * H], F32R, tag=f"wi{k}")
                    nc.sync.dma_start(out=t[:], in_=wi_d[k * 128:(k + 1) * 128, :])
                    wi_t.append(t)
                bib_t = []
                for m in range(GM):
                    t = wpool.tile([128, 1], F32, tag=f"bib{m}", name=f"bib{m}")
                    nc.sync.dma_start(out=t[:], in_=bib_d[m * 128:(m + 1) * 128, :])
                    bib_t.append(t)
                for blk in range(BPC // ABS):
                    s0 = blk * ABS
                    xh, xl = [], []
                    for k in range(KC):
                        th_ = work.tile([128, ABS], F32R, tag=f"xh{k}", name=f"xh{k}")
                        tl_ = work.tile([128, ABS], F32R, tag=f"xl{k}", name=f"xl{k}")
                        xh.append(th_)
                        xl.append(tl_)
                    for sm in range(ABS // 128):
                        xsm = work.tile([128, D], F32, tag="xsm")
                        nc.sync.dma_start(
                            out=xsm[:],
                            in_=x_d[s0 + sm * 128:s0 + (sm + 1) * 128, :])
                        for k in range(KC):
                            ptx = ptpool.tile([128, 128], F32, tag="ptx")
                            nc.tensor.transpose(ptx[:], xsm[:, k * 128:(k + 1) * 128], ident[:])
                            hs_ = xh[k][:, sm * 128:(sm + 1) * 128]
                            nc.vector.tensor_copy(hs_, ptx[:])
                            nc.vector.tensor_sub(xl[k][:, sm * 128:(sm + 1) * 128],
                                                 ptx[:], hs_.bitcast(F32))
                    for m in range(GM):
                        ps = pspool.tile([128, ABS], F32, tag="a1")
                        n = 2 * KC
                        i = 0
                        for k in range(KC):
                            for xt in (xh[k], xl[k]):
                                nc.tensor.matmul(ps[:], wi_t[k][:, m * 128:(m + 1) * 128],
                                                 xt[:], start=(i == 0), stop=(i == n - 1))
                                i += 1
                        ev = work.tile([128, ABS], F32, tag="a1ev")
                        nc.vector.tensor_scalar(ev[:], ps[:], bib_t[m][:], None, ALU.add)
                        nc.sync.dma_start(out=xwi_d[m % KC, m // KC, :, s0:s0 + ABS], in_=ev[:])

            # ================= phase A2: LSTM =================
            with tc.tile_pool(name=R + "whA", bufs=1) as wpool, \
                 tc.tile_pool(name=R + "state", bufs=1) as spool, \
                 tc.tile_pool(name=R + "a2work", bufs=2) as work, \
                 tc.tile_pool(name=R + "a2ps", bufs=4, space="PSUM") as pspool, \
                 tc.tile_pool(name=R + "a2psr", bufs=1, space="PSUM") as psrem, \
                 tc.tile_pool(name=R + "a2psh", bufs=2, space="PSUM") as pshalt:
                wh_t = []
                for k in range(KC):
                    t = wpool.tile([128, 4 * H], F32R, tag=f"wh{k}")
                    nc.sync.dma_start(out=t[:], in_=wh_d[k * 128:(k + 1) * 128, :])
                    wh_t.append(t)

                for blk in range(BPC // NBS):
                    s0 = blk * NBS
                    xw4 = [spool.tile([128, 4, NBS], F32, tag=f"xw4_{k}", name=f"xw4_{k}")
                           for k in range(KC)]
                    for k in range(KC):
                        nc.sync.dma_start(
                            out=xw4[k][:],
                            in_=xwi_d[k, :, :, s0:s0 + NBS].rearrange("g p s -> p g s"))
                    c_t = [spool.tile([128, NBS], F32, tag=f"c{k}", name=f"c{k}") for k in range(KC)]
                    hs_t = [spool.tile([128, NBS], F32, tag=f"hs{k}", name=f"hs{k}") for k in range(KC)]
                    hh_t = [[spool.tile([128, NBS], F32R, tag=f"hh{b}_{k}", name=f"hh{b}_{k}")
                             for k in range(KC)] for b in range(2)]
                    hp = spool.tile([1, NBS], F32, tag="hp")
                    rem = spool.tile([1, NBS], F32, tag="rem")

                    def halt_and_update(step):
                        psh = pshalt.tile([1, NBS], F32, tag="halt")
                        for k in range(KC):
                            nc.tensor.matmul(psh[:], whalt_t[k][:],
                                             hh_t[step % 2][k][:],
                                             start=(k == 0), stop=(k == KC - 1))
                        y = work.tile([1, NBS], F32, tag="y", bufs=1)
                        nc.scalar.activation(y[:], psh[:], AF.Sigmoid, bias=bhalt_t[:])
                        if step == 0:
                            nc.vector.tensor_copy(hp[:], y[:])
                            nc.vector.tensor_scalar(rem[:], y[:], -1.0, 1.0, ALU.mult, ALU.add)
                        else:
                            t2 = work.tile([1, NBS], F32, tag="t2", bufs=1)
                            nc.vector.tensor_scalar(t2[:], hp[:], -1.0, 1.0, ALU.mult, ALU.add)
                            t3 = work.tile([1, NBS], F32, tag="t3", bufs=1)
                            nc.vector.tensor_mul(t3[:], t2[:], y[:])
                            nc.vector.tensor_add(hp[:], hp[:], t3[:])
                            t4 = work.tile([1, NBS], F32, tag="t4", bufs=1)
                            nc.vector.tensor_scalar(t4[:], hp[:], -1.0, 1.0, ALU.mult, ALU.add)
                            nc.vector.tensor_add(rem[:], rem[:], t4[:])

                    # ---- step 0 (h=c=0) ----
                    for k in range(KC):
                        si = work.tile([128, NBS], F32, tag="ga0")
                        nc.scalar.activation(si[:], xw4[k][:, 0, :], AF.Sigmoid)
                        tg = work.tile([128, NBS], F32, tag="ga2")
                        nc.scalar.activation(tg[:], xw4[k][:, 2, :], AF.Tanh)
                        so = work.tile([128, NBS], F32, tag="ga3")
                        nc.scalar.activation(so[:], xw4[k][:, 3, :], AF.Sigmoid)
                        nc.vector.tensor_mul(c_t[k][:], si[:], tg[:])
                        th = work.tile([128, NBS], F32, tag="th")
                        nc.scalar.activation(th[:], c_t[k][:], AF.Tanh)
                        nc.vector.tensor_mul(hh_t[0][k][:], so[:], th[:])
                        nc.vector.tensor_copy(hs_t[k][:], hh_t[0][k][:].bitcast(F32))
                    halt_and_update(0)

                    # ---- steps 1..T-1 ----
                    for step in range(1, T):
                        pb, cb = (step - 1) % 2, step % 2
                        for k in range(KC):
                            gouts = {}
                            for gi, m in ((0, k), (1, 6 + k), (2, 12 + k), (3, 18 + k)):
                                ps = pspool.tile([128, NBS], F32, tag="gps")
                                for kk in range(KC):
                                    nc.tensor.matmul(ps[:], wh_t[kk][:, m * 128:(m + 1) * 128],
                                                     hh_t[pb][kk][:],
                                                     start=(kk == 0), stop=(kk == KC - 1))
                                gp = work.tile([128, NBS], F32, tag=f"gp{gi}")
                                nc.vector.scalar_tensor_tensor(gp[:], ps[:], 1.0, xw4[k][:, gi, :],
                                                               ALU.mult, ALU.add)
                                gact = work.tile([128, NBS], F32, tag=f"ga{gi}")
                                nc.scalar.activation(gact[:], gp[:], AF.Tanh if gi == 2 else AF.Sigmoid)
                                gouts[gi] = gact
                            t1 = work.tile([128, NBS], F32, tag="t1g", bufs=1)
                            nc.vector.tensor_mul(t1[:], gouts[0][:], gouts[2][:])
                            nc.vector.tensor_mul(c_t[k][:], c_t[k][:], gouts[1][:])
                            nc.vector.tensor_add(c_t[k][:], c_t[k][:], t1[:])
                            th = work.tile([128, NBS], F32, tag="th")
                            nc.scalar.activation(th[:], c_t[k][:], AF.Tanh)
                            nc.vector.tensor_mul(hh_t[cb][k][:], gouts[3][:], th[:])
                            nc.vector.tensor_add(hs_t[k][:], hs_t[k][:], hh_t[cb][k][:].bitcast(F32))
                        halt_and_update(step)

                    # ---- avg = rem * hsum / T -> DRAM hi/lo ----
                    psb = psrem.tile([128, NBS], F32, tag="remb")
                    nc.tensor.matmul(psb[:], cst01[:], rem[:], start=True, stop=True)
                    for k in range(KC):
                        av = work.tile([128, NBS], F32, tag="av", bufs=2)
                        nc.vector.tensor_mul(av[:], hs_t[k][:], psb[:])
                        avh = work.tile([128, NBS], F32R, tag="avh", bufs=2)
                        nc.vector.tensor_copy(avh[:], av[:])
                        avl = work.tile([128, NBS], F32R, tag="avl", bufs=2)
                        nc.vector.tensor_sub(avl[:], av[:], avh[:].bitcast(F32))
                        nc.sync.dma_start(out=avgh_d[k * 128:(k + 1) * 128, s0:s0 + NBS], in_=avh[:])
                        nc.sync.dma_start(out=avgl_d[k * 128:(k + 1) * 128, s0:s0 + NBS], in_=avl[:])

            # ================= MoE levels =================
            def moe_level(lv, inh_d, inl_d, split_in, outh_d, outl_d):
                with tc.tile_pool(name=f"{R}mrt{lv}", bufs=1) as rpool:
                    # ---- pass G: gates + top-2 routing, all blocks ----
                    with tc.tile_pool(name=f"{R}mgc{lv}", bufs=1) as mcp, \
                         tc.tile_pool(name=f"{R}mgin{lv}", bufs=2) as ipool, \
                         tc.tile_pool(name=f"{R}mgwk{lv}", bufs=2) as work, \
                         tc.tile_pool(name=f"{R}mgps{lv}", bufs=2, space="PSUM") as pgpool:
                        gateb_lv = mcp.tile([1, E], F32, tag="gb")
                        nc.sync.dma_start(out=gateb_lv[:], in_=gateb_d[lv, :, :])
                        gw_lv = []
                        for k in range(KC):
                            t = mcp.tile([128, E], F32R, tag=f"gw{k}", name=f"gw{k}")
                            nc.sync.dma_start(out=t[:], in_=gate_d[lv, k * 128:(k + 1) * 128, :])
                            gw_lv.append(t)
                        w4n_all = []
                        for blk in range(BPC // GBS):
                            s0 = blk * GBS
                            in6h = ipool.tile([128, KC, GBS], F32R, tag="in6h")
                            nc.sync.dma_start(
                                out=in6h[:],
                                in_=inh_d[:, s0:s0 + GBS].rearrange("(k p) s -> p k s", p=128))
                            in6l = ipool.tile([128, KC, GBS], F32R, tag="in6l")
                            nc.sync.dma_start(
                                out=in6l[:],
                                in_=inl_d[:, s0:s0 + GBS].rearrange("(k p) s -> p k s", p=128))
                            w4n = rpool.tile([E, GBS], F32, tag=f"w4n{blk}", name=f"w4n{blk}")
                            for sm in range(GBS // 128):
                                pg = pgpool.tile([128, E], F32, tag="pg")
                                for k in range(KC):
                                    nc.tensor.matmul(pg[:], in6h[:, k, sm * 128:(sm + 1) * 128],
                                                     gw_lv[k][:], start=(k == 0), stop=False)
                                    nc.tensor.matmul(pg[:], in6l[:, k, sm * 128:(sm + 1) * 128],
                                                     gw_lv[k][:], start=False, stop=False)
                                nc.tensor.matmul(pg[:], ones1[:], gateb_lv[:], start=False, stop=True)
                                lt = work.tile([128, E], F32, tag="lt")
                                nc.vector.tensor_copy(lt[:], pg[:])
                                m1 = work.tile([128, 1], F32, tag="m1")
                                nc.vector.tensor_reduce(m1[:], lt[:], AX.X, ALU.max)
                                eq1 = work.tile([128, E], F32, tag="eq1")
                                nc.vector.tensor_scalar(eq1[:], lt[:], m1[:], None, ALU.is_equal)
                                msk = work.tile([128, E], F32, tag="msk")
                                nc.vector.scalar_tensor_tensor(msk[:], eq1[:], -1e30, lt[:], ALU.mult, ALU.add)
                                m2 = work.tile([128, 1], F32, tag="m2")
                                nc.vector.tensor_reduce(m2[:], msk[:], AX.X, ALU.max)
                                dd = work.tile([128, 1], F32, tag="dd")
                                nc.vector.tensor_sub(dd[:], m1[:], m2[:])
                                sg = work.tile([128, 1], F32, tag="sg")
                                nc.scalar.activation(sg[:], dd[:], AF.Sigmoid)
                                eq2 = work.tile([128, E], F32, tag="eq2")
                                nc.vector.tensor_scalar(eq2[:], msk[:], m2[:], None, ALU.is_equal)
                                oms = work.tile([128, 1], F32, tag="oms")
                                nc.vector.tensor_scalar(oms[:], sg[:], -1.0, 1.0, ALU.mult, ALU.add)
                                wa = work.tile([128, E], F32, tag="wa")
                                nc.vector.tensor_scalar(wa[:], eq1[:], sg[:], None, ALU.mult)
                                wb = work.tile([128, E], F32, tag="wb")
                                nc.vector.tensor_scalar(wb[:], eq2[:], oms[:], None, ALU.mult)
                                wf = work.tile([128, E], F32, tag="wf")
                                nc.vector.tensor_add(wf[:], wa[:], wb[:])
                                ptp = pgpool.tile([E, 128], F32, tag="ptp")
                                nc.tensor.transpose(ptp[:], wf[:], ident[:])
                                nc.vector.tensor_copy(w4n[:, sm * 128:(sm + 1) * 128], ptp[:])
                            w4n_all.append(w4n)

                    # ---- pass E: experts, PSUM per expert, SBUF accumulate ----
                    with tc.tile_pool(name=f"{R}mw{lv}", bufs=1) as wpool, \
                         tc.tile_pool(name=f"{R}mec{lv}", bufs=1) as mcp, \
                         tc.tile_pool(name=f"{R}mein{lv}", bufs=1) as ipool, \
                         tc.tile_pool(name=f"{R}mewk{lv}", bufs=2) as work, \
                         tc.tile_pool(name=f"{R}mhw{lv}", bufs=1) as hpool, \
                         tc.tile_pool(name=f"{R}macc{lv}", bufs=1) as apool, \
                         tc.tile_pool(name=f"{R}meps{lv}", bufs=3, space="PSUM") as pspool, \
                         tc.tile_pool(name=f"{R}mepo{lv}", bufs=2, space="PSUM") as popool, \
                         tc.tile_pool(name=f"{R}mepw{lv}", bufs=2, space="PSUM") as pwpool:
                        b1_lv = []
                        for e in range(E):
                            ts = []
                            for k in range(KC):
                                t = mcp.tile([128, 1], F32, tag=f"b1_{e}_{k}", name=f"b1_{e}_{k}")
                                nc.sync.dma_start(out=t[:], in_=b1_d[lv, e, k * 128:(k + 1) * 128, :])
                                ts.append(t)
                            b1_lv.append(ts)
                        b2_lv = []
                        for m in range(KC):
                            t = mcp.tile([E, 128], F32, tag=f"b2_{m}", name=f"b2_{m}")
                            nc.sync.dma_start(out=t[:], in_=b2s_d[lv, :, m * 128:(m + 1) * 128])
                            b2_lv.append(t)
                        w1_t, w2_t = [], []
                        for e in range(E):
                            k1, k2 = [], []
                            for k in range(KC):
                                t1 = wpool.tile([128, H], F32R, tag=f"w1_{e}_{k}", name=f"w1_{e}_{k}")
                                nc.sync.dma_start(out=t1[:], in_=w1_d[lv, e, k * 128:(k + 1) * 128, :])
                                k1.append(t1)
                                t2 = wpool.tile([128, D], F32R, tag=f"w2_{e}_{k}", name=f"w2_{e}_{k}")
                                nc.sync.dma_start(out=t2[:], in_=w2_d[lv, e, k * 128:(k + 1) * 128, :])
                                k2.append(t2)
                            w1_t.append(k1)
                            w2_t.append(k2)

                        for blk in range(BPC // MBS):
                            s0 = blk * MBS
                            g0 = s0 // GBS
                            goff = s0 - g0 * GBS
                            in6h = ipool.tile([128, KC, MBS], F32R, tag="in6h")
                            nc.sync.dma_start(
                                out=in6h[:],
                                in_=inh_d[:, s0:s0 + MBS].rearrange("(k p) s -> p k s", p=128))
                            ins = [in6h]
                            if split_in:
                                in6l = ipool.tile([128, KC, MBS], F32R, tag="in6l")
                                nc.sync.dma_start(
                                    out=in6l[:],
                                    in_=inl_d[:, s0:s0 + MBS].rearrange("(k p) s -> p k s", p=128))
                                ins.append(in6l)
                            acc = [apool.tile([128, MBS], F32, tag=f"acc{m}", name=f"acc{m}")
                                   for m in range(KC)]
                            for e in range(E):
                                we_row = work.tile([1, MBS], F32, tag="we_row")
                                nc.sync.dma_start(out=we_row[:],
                                                  in_=w4n_all[g0][e:e + 1, goff:goff + MBS])
                                pwb = pwpool.tile([128, MBS], F32, tag="pwb")
                                nc.tensor.matmul(pwb[:], ones1[:], we_row[:], start=True, stop=True)
                                hw = []
                                for m in range(KC):
                                    ph = pspool.tile([128, MBS], F32, tag="ph1")
                                    n = KC * len(ins)
                                    i = 0
                                    for k in range(KC):
                                        for it in ins:
                                            nc.tensor.matmul(ph[:], w1_t[e][k][:, m * 128:(m + 1) * 128],
                                                             it[:, k, :],
                                                             start=(i == 0), stop=(i == n - 1))
                                            i += 1
                                    hm = work.tile([128, MBS], F32, tag="hm")
                                    nc.scalar.activation(hm[:], ph[:], AF.Relu, bias=b1_lv[e][m][:])
                                    if split_in:
                                        hf = work.tile([128, MBS], F32, tag="hf")
                                        nc.vector.tensor_mul(hf[:], hm[:], pwb[:])
                                        hwh = hpool.tile([128, MBS], F32R, tag=f"hwh{m}", name=f"hwh{m}")
                                        nc.vector.tensor_copy(hwh[:], hf[:])
                                        hwl = hpool.tile([128, MBS], F32R, tag=f"hwl{m}", name=f"hwl{m}")
                                        nc.vector.tensor_sub(hwl[:], hf[:], hwh[:].bitcast(F32))
                                        hw.append((hwh, hwl))
                                    else:
                                        hwh = hpool.tile([128, MBS], F32R, tag=f"hwh{m}", name=f"hwh{m}")
                                        nc.vector.tensor_mul(hwh[:], hm[:], pwb[:])
                                        hw.append((hwh,))
                                for m in range(KC):
                                    po = popool.tile([128, MBS], F32, tag="po")
                                    if e == 0:
                                        nc.tensor.matmul(po[:], b2_lv[m][:],
                                                         w4n_all[g0][:, goff:goff + MBS],
                                                         start=True, stop=False)
                                    n = KC * len(ins)
                                    i = 0
                                    for k in range(KC):
                                        for hx in hw[k]:
                                            nc.tensor.matmul(po[:], w2_t[e][k][:, m * 128:(m + 1) * 128],
                                                             hx[:],
                                                             start=(e != 0 and i == 0),
                                                             stop=(i == n - 1))
                                            i += 1
                                    if e == 0:
                                        nc.vector.tensor_copy(acc[m][:], po[:])
                                    else:
                                        nc.vector.tensor_add(acc[m][:], acc[m][:], po[:])
                            for m in range(KC):
                                if outl_d is None:
                                    nc.sync.dma_start(out=outh_d[m * 128:(m + 1) * 128, s0:s0 + MBS],
                                                      in_=acc[m][:])
                                else:
                                    ach = work.tile([128, MBS], F32R, tag="ach")
                                    nc.vector.tensor_copy(ach[:], acc[m][:])
                                    acl = work.tile([128, MBS], F32R, tag="acl")
                                    nc.vector.tensor_sub(acl[:], acc[m][:], ach[:].bitcast(F32))
                                    nc.sync.dma_start(out=outh_d[m * 128:(m + 1) * 128, s0:s0 + MBS],
                                                      in_=ach[:])
                                    nc.sync.dma_start(out=outl_d[m * 128:(m + 1) * 128, s0:s0 + MBS],
                                                      in_=acl[:])

            moe_level(0, avgh_d, avgl_d, True, l0h_d, l0l_d)
            moe_level(1, l0h_d, l0l_d, False, l1o_d, None)

            # ================= LayerNorm + transpose out =================
            with tc.tile_pool(name=R + "ln", bufs=2) as work, \
                 tc.tile_pool(name=R + "lnc", bufs=1) as lcp, \
                 tc.tile_pool(name=R + "lnps", bufs=1, space="PSUM") as pspool, \
                 tc.tile_pool(name=R + "lnpst", bufs=2, space="PSUM") as pst, \
                 tc.tile_pool(name=R + "lnpsb", bufs=1, space="PSUM") as psb, \
                 tc.tile_pool(name=R + "outp", bufs=2) as opool:
                LBS = 512
                gam_t, bet_t = [], []
                for k in range(KC):
                    g = lcp.tile([128, 1], F32, tag=f"gam{k}", name=f"gam{k}")
                    nc.sync.dma_start(out=g[:], in_=gamma_d[k * 128:(k + 1) * 128, :])
                    gam_t.append(g)
                    bb = lcp.tile([128, 1], F32, tag=f"bet{k}", name=f"bet{k}")
                    nc.sync.dma_start(out=bb[:], in_=beta_d[k * 128:(k + 1) * 128, :])
                    bet_t.append(bb)
                for blk in range(BPC // LBS):
                    s0 = blk * LBS
                    y6 = work.tile([128, KC, LBS], F32, tag="y6")
                    nc.sync.dma_start(
                        out=y6[:],
                        in_=l1o_d[:, s0:s0 + LBS].rearrange("(k p) s -> p k s", p=128))
                    pmu = pspool.tile([1, LBS], F32, tag="pmu")
                    for k in range(KC):
                        nc.tensor.matmul(pmu[:], oinv[:], y6[:, k, :], start=(k == 0), stop=(k == KC - 1))
                    mu = work.tile([1, LBS], F32, tag="mu")
                    nc.vector.tensor_copy(mu[:], pmu[:])
                    pmub = psb.tile([128, LBS], F32, tag="pmub")
                    nc.tensor.matmul(pmub[:], ones1[:], mu[:], start=True, stop=True)
                    dts = []
                    pvar = pspool.tile([1, LBS], F32, tag="pvar")
                    for k in range(KC):
                        d_ = work.tile([128, LBS], F32, tag=f"d{k}", name=f"d{k}")
                        nc.vector.tensor_sub(d_[:], y6[:, k, :], pmub[:])
                        dts.append(d_)
                        q = work.tile([128, LBS], F32, tag="q")
                        nc.vector.tensor_mul(q[:], d_[:], d_[:])
                        nc.tensor.matmul(pvar[:], oinv[:], q[:], start=(k == 0), stop=(k == KC - 1))
                    sd = work.tile([1, LBS], F32, tag="sd")
                    nc.scalar.activation(sd[:], pvar[:], AF.Sqrt, bias=eps_t[:])
                    rs = work.tile([1, LBS], F32, tag="rs")
                    nc.vector.reciprocal(rs[:], sd[:])
                    prsb = psb.tile([128, LBS], F32, tag="prsb")
                    nc.tensor.matmul(prsb[:], ones1[:], rs[:], start=True, stop=True)
                    for k in range(KC):
                        nc.vector.scalar_tensor_tensor(dts[k][:], dts[k][:], gam_t[k][:], prsb[:],
                                                       ALU.mult, ALU.mult)
                        nc.vector.tensor_scalar(dts[k][:], dts[k][:], bet_t[k][:], None, ALU.add)
                    for sm in range(LBS // 128):
                        obig = opool.tile([128, D], F32, tag="obig")
                        for k in range(KC):
                            ptt = pst.tile([128, 128], F32, tag="ptt")
                            nc.tensor.transpose(ptt[:], dts[k][:, sm * 128:(sm + 1) * 128], ident[:])
                            nc.vector.tensor_copy(obig[:, k * 128:(k + 1) * 128], ptt[:])
                        nc.sync.dma_start(out=out_d[s0 + sm * 128:s0 + (sm + 1) * 128, :], in_=obig[:])

    nc.compile()
    return nc, names


def prep_shared(inputs):
    """Host-side weight prep shared across cores (raw fp32, transposed)."""
    f32 = lambda a: np.ascontiguousarray(a, np.float32)

    def packT(w):
        return np.ascontiguousarray(f32(w).T)

    d = {}
    d["wiT"] = packT(inputs["Wi"])
    d["whT"] = packT(inputs["Wh"])
    d["whaltT"] = packT(inputs["Whalt"])
    d["bib"] = (f32(inputs["bi"]) + f32(inputs["bh"]))[:, None]
    d["bhalt"] = f32(inputs["bhalt"])[:, None]
    d["gateT"] = np.stack([packT(inputs["gateW"][l]) for l in range(LV)])
    d["gateb"] = f32(inputs["gateb"])[:, None, :]
    d["w1T"] = np.stack([np.stack([packT(inputs["W1"][l, e]) for e in range(E)])
                         for l in range(LV)])
    d["w2T"] = np.stack([np.stack([packT(inputs["W2"][l, e]) for e in range(E)])
                         for l in range(LV)])
    d["b1"] = f32(inputs["b1"])[..., None]
    d["b2s"] = f32(inputs["b2"])
    d["gamma"] = f32(inputs["gamma"])[:, None]
    d["beta"] = f32(inputs["beta"])[:, None]
    d["ident128"] = np.eye(128, dtype=np.float32)
    return d


# ---------------------------------------------------------------------------
# Runner: cached jitted shard_map executable; weights replicated (P(None)),
# x / out sharded over the 8 cores (P("core")).
# ---------------------------------------------------------------------------
_STATE = {}


def _get_exec(reps=1):
    key = ("exec", reps)
    if key in _STATE:
        return _STATE[key]
    import jax
    from jax.sharding import Mesh, PartitionSpec as P, NamedSharding
    from jax.experimental.shard_map import shard_map
    from concourse import bass2jax
    from concourse.bass2jax import install_neuronx_cc_hook, _bass_exec_p

    nc, names = build(reps)
    install_neuronx_cc_hook()

    partition_name = nc.partition_id_tensor.name if nc.partition_id_tensor else None
    in_names, out_names, out_avals = [], [], []
    for alloc in nc.m.functions[0].allocations:
        if not isinstance(alloc, mybir.MemoryLocationSet):
            continue
        if not alloc.memorylocations:
            continue
        name = alloc.memorylocations[0].name
        if alloc.kind == "ExternalInput":
            if name != partition_name:
                in_names.append(name)
        elif alloc.kind == "ExternalOutput":
            out_names.append(name)
            shape = tuple(alloc.tensor_shape)
            dtype = mybir.dt.np(alloc.dtype)
            out_avals.append(jax.core.ShapedArray(shape, dtype))
    n_params = len(in_names)
    n_outs = len(out_avals)
    all_in_names = list(in_names) + list(out_names)
    if partition_name is not None:
        all_in_names.append(partition_name)

    def _body(*args):
        operands = list(args)
        if partition_name is not None:
            operands.append(bass2jax.partition_id_tensor())
        outs = _bass_exec_p.bind(
            *operands,
            out_avals=tuple(out_avals),
            in_names=tuple(all_in_names),
            out_names=tuple(out_names),
            lowering_input_output_aliases=(),
            sim_require_finite=True,
            sim_require_nnan=True,
            nc=nc,
        )
        return tuple(outs)

    import jax.numpy as jnp
    devices = jax.devices()[:NCORES]
    mesh = Mesh(np.asarray(devices), ("core",))
    in_specs = tuple(P("core") if n == "x" else P(None) for n in in_names) \
        + (P("core"),) * n_outs
    out_specs = (P("core"),) * n_outs
    donate = tuple(range(n_params, n_params + n_outs))
    sharded = jax.jit(
        shard_map(_body, mesh=mesh, in_specs=in_specs, out_specs=out_specs,
                  check_rep=False),
        donate_argnums=donate, keep_unused=True)

    out_global_shapes = [(NCORES * a.shape[0],) + tuple(a.shape[1:]) for a in out_avals]
    out_shardings = [NamedSharding(mesh, P("core"))] * n_outs
    zeros_fns = [jax.jit(lambda s=s, d=a.dtype: jnp.zeros(s, d), out_shardings=sh)
                 for s, a, sh in zip(out_global_shapes, out_avals, out_shardings)]
    x_sharding = NamedSharding(mesh, P("core"))
    w_sharding = NamedSharding(mesh, P(None))

    st = dict(nc=nc, names=names, sharded=sharded, in_names=in_names,
              out_names=out_names, zeros_fns=zeros_fns,
              x_sharding=x_sharding, w_sharding=w_sharding, mesh=mesh)
    _STATE[key] = st
    return st


def _device_args(st, inputs):
    """Return the positional device args (weights cached across calls)."""
    import jax
    shared = None
    args = []
    wcache = _STATE.setdefault("wcache", {})
    for nm in st["in_names"]:
        if nm == "x":
            x = np.ascontiguousarray(inputs["x"], np.float32)
            args.append(jax.device_put(x, st["x_sharding"]))
            continue
        if shared is None:
            shared = prep_shared(inputs)
        host = shared[nm]
        ent = wcache.get(nm)
        if ent is not None and ent[0].shape == host.shape and np.array_equal(ent[0], host):
            args.append(ent[1])
        else:
            dev = jax.device_put(host, st["w_sharding"])
            wcache[nm] = (host, dev)
            args.append(dev)
    return args


def kernel(**inputs):
    st = _get_exec()
    args = _device_args(st, inputs)
    zeros = [f() for f in st["zeros_fns"]]
    outs = st["sharded"](*args, *zeros)
    out = np.asarray(outs[st["out_names"].index("out")])
    return out.astype(np.float32)


def bench(inputs, iters=5, reps=1):
    """Time repeat executions with device-resident inputs.

    reps>1 builds a variant NEFF that executes the whole computation `reps`
    times back-to-back; the t(reps)-t(1) delta isolates true kernel time
    from the ~70ms axon dispatch floor.
    """
    import time
    st = _get_exec(reps)
    args = _device_args(st, inputs)
    times = []
    for _ in range(iters + 1):
        zeros = [f() for f in st["zeros_fns"]]
        for z in zeros:
            z.block_until_ready()
        t0 = time.perf_counter()
        outs = st["sharded"](*args, *zeros)
        for o in outs:
            o.block_until_ready()
        times.append(time.perf_counter() - t0)
    return times[1:]


if __name__ == "__main__":
    import time
    t0 = time.time()
    nc, names = build()
    print("build+compile ok in", time.time() - t0, "s")
